# revision 10
# baseline (speedup 1.0000x reference)
"""Trainium2 Bass kernel for nn_CrossAttention (B=8, N=M=2048, C=512, H=4).

Sharding: data-parallel over batch - one batch element per NeuronCore (8 cores).

v4 design (v3 baseline 228.8us):
  - The 8-core run trips the board GPIO power throttle at ~65us (PE drops
    2.4->~1.95GHz).  v4 cuts total engine activity: gpsimd is eliminated
    entirely (its partition_all_reduce was 58.6us busy/core).
  - Softmax denominator chain per combo: PE ones-matmul column-sum of esE
    into a [1,SW] psum row (512 cyc), DVE reciprocal_approx_fast on the row,
    DMA partition-broadcast of the recip row to [P,SW] (idle DMA engines),
    DVE mul.  Chain k is emitted spread over combo k+1 (j2..j5 slots).
  - pv matmuls run at lag-2 behind the exp (deque), killing the ~300ns
    head-of-queue waits on ACT seen each j in the v3 trace; pairs 6,7 of
    combo k spill into combo k+1's first two j-slots.
  - out-proj weave items moved to j4/j6 slots (one per slot) so the aux
    psum ring (bufs=2) never stalls PE on a back-to-back pair.
  - DMA issue is spread across engine queues (sync: W + FT2 stripe 0 first;
    gpsimd queue: FT1 + FT2 s1-3 + Wp) - v3 serialized 60 issues at ~620ns
    on sync, costing ~9us of startup idle.

Engine budget/core (throttled): PE ~193us busy (pacer), ACT ~154us, DVE ~130us.
If the GPIO throttle lifts with gpsimd gone: PE ~160us.
"""
import sys
from collections import deque

for _p in ("/opt/trn_rl_repo", "/root/.axon_site/_ro/trn_rl_repo"):
    if _p not in sys.path:
        sys.path.insert(0, _p)

import numpy as np
import concourse.bass as bass
import concourse.bacc as bacc
import concourse.tile as tile
from concourse import mybir
from concourse.bass_utils import run_bass_kernel_spmd

F32 = mybir.dt.float32
F16 = mybir.dt.float16
EXP = mybir.ActivationFunctionType.Exp
IDENT = mybir.ActivationFunctionType.Identity

B, N, M, C = 8, 2048, 2048, 512
H, D = 4, 128
SCALE = 1.0 / np.sqrt(C)
P = 128
NB = N // P        # 16 n-blocks
MB = M // P        # 16 m-blocks
KC = C // P        # 4 contraction chunks (also = heads since D=128)
NS = 4             # n-stripes of 512
SW = N // NS       # stripe width 512

# denominator partition-reduction: "pedma" = PE reduce + DMA broadcast
# (gpsimd-free); "gpsimd" = v3's partition_all_reduce fallback
DN_MODE = "pedma"


def build_nc():
    nc = bacc.Bacc(None, target_bir_lowering=False)
    dF1T = nc.dram_tensor("F1T", [C, N], F16, kind="ExternalInput")
    dF2T = nc.dram_tensor("F2T", [C, M], F16, kind="ExternalInput")
    dW = nc.dram_tensor("Wqkv", [C, C], F16, kind="ExternalInput")
    dBqc = nc.dram_tensor("bqc", [P, KC], F32, kind="ExternalInput")
    dWp = nc.dram_tensor("Wproj", [C, C], F16, kind="ExternalInput")
    dBp = nc.dram_tensor("bproj", [1, C], F32, kind="ExternalInput")
    dOut = nc.dram_tensor("OUT", [N, C], F32, kind="ExternalOutput")

    d_ones_col = nc.inline_tensor(np.ones((P, 1), np.float16), name="ones_col")
    d_ident16 = nc.inline_tensor(np.eye(P, dtype=np.float16), name="identity16")

    with tile.TileContext(nc) as tc:
        with (
            tc.tile_pool(name="const", bufs=1) as const,
            tc.tile_pool(name="persist", bufs=1) as persist,
            tc.tile_pool(name="ftp", bufs=1) as ftp,
        ):
            # ---- DMA issue split across engine queues, ONE descriptor per
            # stripe: a [C,*] DRAM tensor maps to a [P, KC, *] SBUF tile via
            # a 3D access pattern, so all 4 kc-chunks land in one issue
            # (~620ns each on the queue; v4 serialized 4x as many). ----
            Wt = const.tile([P, KC, C], F16, name="Wt")
            W = [Wt[:, kc, :] for kc in range(KC)]
            F1t = ftp.tile([P, KC, N], F16, name="F1t")
            FT1 = [F1t[:, kc, :] for kc in range(KC)]
            F2t = ftp.tile([P, KC, M], F16, name="F2t")
            FT2 = [F2t[:, kc, :] for kc in range(KC)]

            def chunked_dram(dt_, width):
                # [C, width] dram AP -> [P, KC, width] (partition-major)
                return dt_.rearrange("(kc p) w -> p kc w", kc=KC, p=P)

            dWv = chunked_dram(dW[:, :], C)
            dF2v = chunked_dram(dF2T[:, :], M)
            dF1v = chunked_dram(dF1T[:, :], N)
            nc.sync.dma_start(Wt, dWv)
            nc.sync.dma_start(F2t[:, :, 0:SW], dF2v[:, :, 0:SW])
            bq_col = const.tile([P, KC], F32)
            nc.sync.dma_start(bq_col, dBqc[:])
            ident16 = const.tile([P, P], F16)
            nc.sync.dma_start(ident16, d_ident16[:])
            nc.sync.dma_start(F2t[:, :, SW:2 * SW], dF2v[:, :, SW:2 * SW])
            ones_col = const.tile([P, 1], F16)
            nc.sync.dma_start(ones_col, d_ones_col[:])
            bp_row = const.tile([1, C], F32)
            nc.sync.dma_start(bp_row, dBp[:])

            # later-needed loads on the gpsimd queue (idle until main loop);
            # bp_bcast last so its wait on bp_row doesn't block the queue
            for g in (2, 3):
                nc.gpsimd.dma_start(
                    F2t[:, :, g * SW:(g + 1) * SW], dF2v[:, :, g * SW:(g + 1) * SW]
                )
            for g in range(NS):
                nc.gpsimd.dma_start(
                    F1t[:, :, g * SW:(g + 1) * SW], dF1v[:, :, g * SW:(g + 1) * SW]
                )
            Wpt = const.tile([P, KC, C], F16, name="Wpt")
            Wp = [Wpt[:, kc, :] for kc in range(KC)]
            nc.gpsimd.dma_start(Wpt, chunked_dram(dWp[:, :], C))
            bp_bcast = const.tile([P, C], F32)
            nc.gpsimd.partition_broadcast(bp_bcast, bp_row)

            # ---- persistent activations ----
            qT = [persist.tile([P, N], F16, name=f"qT{i}") for i in range(KC)]
            kvT = [persist.tile([P, M], F16, name=f"kvT{i}") for i in range(KC)]
            kvn = [persist.tile([P, C], F16, name=f"kvn{i}") for i in range(MB)]

            # ---- prefix: kvT projections + kvn transposes (dense PE) ----
            with tc.tile_pool(name="pfps", bufs=8, space="PSUM") as pfps:
                for g in range(NS):
                    # kvT stripe g for all 4 output chunks
                    for co in range(KC):
                        pj = pfps.tile([P, SW], F32, tag="pj", bufs=4)
                        for kc in range(KC):
                            nc.tensor.matmul(
                                pj,
                                W[kc][:, co * P:(co + 1) * P],
                                FT2[kc][:, g * SW:(g + 1) * SW],
                                start=(kc == 0),
                                stop=(kc == KC - 1),
                            )
                        # evac on ACT (idle in prefix): kvT = pj + bq
                        nc.scalar.activation(
                            kvT[co][:, g * SW:(g + 1) * SW],
                            pj,
                            IDENT,
                            bias=bq_col[:, co:co + 1],
                        )
                    # kvn for this stripe's 4 m-blocks
                    for mb in range(4 * g, 4 * g + 4):
                        pjt = pfps.tile([P, C], F16, tag="pjt", bufs=2)
                        for hh in range(H):
                            nc.tensor.transpose(
                                pjt[:, hh * P:(hh + 1) * P],
                                kvT[hh][:, mb * P:(mb + 1) * P],
                                ident16,
                            )
                        nc.vector.tensor_copy(kvn[mb], pjt)

            # ---- attention + weaved qT projections + weaved out-proj ----
            with (
                tc.tile_pool(name="xtp", bufs=1) as xtp,
                tc.tile_pool(name="et", bufs=2) as epool,
                tc.tile_pool(name="es", bufs=2) as espool,
                tc.tile_pool(name="scps", bufs=2, space="PSUM") as scps,
                tc.tile_pool(name="pvps", bufs=2, space="PSUM") as pvps,
                tc.tile_pool(name="auxps", bufs=2, space="PSUM") as auxps,
                tc.tile_pool(name="sm", bufs=2) as sm,
                tc.tile_pool(name="osb", bufs=3) as osb,
            ):
                xT = [xtp.tile([P, N], F16, name=f"xT{i}") for i in range(KC)]

                def emit_qT_proj(co, g):
                    pj = auxps.tile([P, SW], F32, tag="aux")
                    for kc in range(KC):
                        nc.tensor.matmul(
                            pj,
                            W[kc][:, co * P:(co + 1) * P],
                            FT1[kc][:, g * SW:(g + 1) * SW],
                            start=(kc == 0),
                            stop=(kc == KC - 1),
                        )
                    nc.vector.tensor_scalar_add(
                        qT[co][:, g * SW:(g + 1) * SW],
                        pj,
                        bq_col[:, co:co + 1],
                    )

                def emit_ph4_start(nb, nchunks=KC):
                    pr = auxps.tile([P, C], F32, tag="aux", name="pr")
                    for kc in range(nchunks):
                        nc.tensor.matmul(
                            pr,
                            xT[kc][:, nb * P:(nb + 1) * P],
                            Wp[kc],
                            start=(kc == 0),
                            stop=(kc == KC - 1),
                        )
                    return pr

                def emit_ph4_finish(nb, pr, kc0=KC):
                    for kc in range(kc0, KC):
                        nc.tensor.matmul(
                            pr,
                            xT[kc][:, nb * P:(nb + 1) * P],
                            Wp[kc],
                            start=False,
                            stop=(kc == KC - 1),
                        )
                    ot = osb.tile([P, C], F32, tag="ot")
                    nc.vector.tensor_add(ot, pr, bp_bcast)
                    nc.sync.dma_start(dOut[nb * P:(nb + 1) * P, :], ot)

                def emit_ph4_nb(nb):
                    emit_ph4_finish(nb, emit_ph4_start(nb))

                combos = [(s, h) for s in range(NS) for h in range(H)]
                # qT-proj weave (j1 slot): combo k emits combo k+1's qT
                qt_sched = [None] * 16
                for k in range(15):
                    qt_sched[k] = combos[k + 1]
                # out-proj weave: stripe s's 4 blocks at combos 4(s+1)+1
                # (j4+j6) and 4(s+1)+2 (j4+j6); stripe 3 in the tail
                op_sched = [[] for _ in range(16)]
                for s in range(NS - 1):
                    for i, nb in enumerate(range(4 * s, 4 * s + 4)):
                        op_sched[4 * (s + 1) + 1 + i // 2].append(nb)

                # deferred normalize chain state from the previous combo
                pending = {}

                def chain_reduce(pp):
                    if DN_MODE == "gpsimd":
                        nc.gpsimd.partition_all_reduce(
                            pp["dnb"], pp["esE"], channels=P,
                            reduce_op=bass.bass_isa.ReduceOp.add,
                        )
                        return
                    ct = auxps.tile([P, C], F32, tag="aux")
                    pp["chain"] = ct
                    nc.tensor.matmul(
                        ct[0:1, 0:SW], ones_col, pp["esE"],
                        start=True, stop=True,
                    )

                def chain_recip(pp):
                    if DN_MODE == "gpsimd":
                        nc.vector.reciprocal_approx_fast(pp["recip"], pp["dnb"])
                        return
                    nc.vector.reciprocal_approx_fast(
                        pp["rrow"], pp["chain"][0:1, 0:SW]
                    )

                def chain_bcast(pp):
                    if DN_MODE == "gpsimd":
                        return
                    # small gpsimd op (~0.7us): 16x cheaper than v3's
                    # partition_all_reduce of the full [P,SW] tile
                    nc.gpsimd.partition_broadcast(pp["bcast"], pp["rrow"])

                def chain_mul(pp):
                    s, h = pp["sh"]
                    mulin = pp["recip"] if DN_MODE == "gpsimd" else pp["bcast"]
                    with nc.allow_low_precision(
                        reason="x values O(0.1); fp16 keeps 5e-4 rel"
                    ):
                        nc.vector.tensor_mul(
                            xT[h][:, s * SW:(s + 1) * SW],
                            pp["pv"], mulin,
                        )

                pvq = deque()
                emit_qT_proj(0, 0)  # combo 0's qT, ahead of the loop

                for k, (s, h) in enumerate(combos):
                    E = epool.tile([P, MB, SW], F16, tag="E")
                    pv = pvps.tile([P, SW], F32, tag="pv")

                    def pv_pair(jj, E=E, pv=pv, h=h):
                        for mb in (2 * jj, 2 * jj + 1):
                            nc.tensor.matmul(
                                pv,
                                kvn[mb][:, h * P:(h + 1) * P],
                                E[:, mb, :],
                                start=(mb == 0),
                                stop=(mb == MB - 1),
                            )

                    esA = espool.tile([P, 4, SW], F16, tag="esA")
                    esB = espool.tile([P, 4, SW], F16, tag="esB")
                    esC = espool.tile([P, 4, SW], F16, tag="esC")
                    esD = espool.tile([P, 2, SW], F16, tag="esD")
                    esE = espool.tile([P, SW], F16, tag="esE")
                    for j in range(MB // 2):
                        sc = scps.tile([P, 2, SW], F32, tag="sc")
                        for i in range(2):
                            mb = 2 * j + i
                            nc.tensor.matmul(
                                sc[:, i, :],
                                kvT[h][:, mb * P:(mb + 1) * P],
                                qT[h][:, s * SW:(s + 1) * SW],
                                start=True,
                                stop=True,
                            )
                        nc.scalar.activation(
                            E[:, 2 * j:2 * j + 2, :].rearrange(
                                "p a b -> p (a b)"
                            ),
                            sc.rearrange("p a b -> p (a b)"),
                            EXP,
                            scale=float(SCALE),
                        )
                        pvq.append(lambda jj=j, pf=pv_pair: pf(jj))
                        if len(pvq) > 2:
                            pvq.popleft()()
                        # weave + deferred chain of the previous combo.
                        # progressive denominator tree: esA@j4, +E[8:12]@j6,
                        # +E[12:16] after the loop, so esE trails the last
                        # exp by only ~2 DVE ops (the v4 bulk tree made the
                        # next combo's chain_reduce stall the PE queue head).
                        if j == 1 and qt_sched[k] is not None:
                            g2, h2 = qt_sched[k][0], qt_sched[k][1]
                            emit_qT_proj(h2, g2)
                        elif j == 3 and pending:
                            chain_reduce(pending)
                        elif j == 4:
                            if pending:
                                chain_recip(pending)
                                chain_bcast(pending)
                            if op_sched[k]:
                                emit_ph4_nb(op_sched[k][0])
                            with nc.allow_low_precision(
                                reason="fp16 partial sums of E for softmax "
                                "denominator; ~1e-3 rel"
                            ):
                                nc.vector.tensor_add(
                                    esA, E[:, 0:4, :], E[:, 4:8, :]
                                )
                        elif j == 5 and pending:
                            chain_mul(pending)
                        elif j == 6:
                            with nc.allow_low_precision(
                                reason="fp16 partial sums of E for softmax "
                                "denominator; ~1e-3 rel"
                            ):
                                nc.vector.tensor_add(esB, esA, E[:, 8:12, :])
                            if len(op_sched[k]) > 1:
                                emit_ph4_nb(op_sched[k][1])
                    with nc.allow_low_precision(
                        reason="fp16 partial sums of E for softmax "
                        "denominator; ~1e-3 rel"
                    ):
                        nc.vector.tensor_add(esC, esB, E[:, 12:16, :])
                        nc.vector.tensor_add(esD, esC[:, 0:2, :], esC[:, 2:4, :])
                        nc.vector.tensor_add(esE, esD[:, 0, :], esD[:, 1, :])
                    pending = {"sh": (s, h), "pv": pv, "esE": esE}
                    if DN_MODE == "gpsimd":
                        pending["dnb"] = sm.tile(
                            [P, SW], F32, tag="dnb", name="dnb"
                        )
                        pending["recip"] = sm.tile(
                            [P, SW], F32, tag="recip", name="recip"
                        )
                    else:
                        pending["rrow"] = sm.tile(
                            [1, SW], F32, tag="rrow", name="rrow"
                        )
                        pending["bcast"] = sm.tile(
                            [P, SW], F32, tag="bcast", name="bcast"
                        )

                # ---- tail: drain pv, last chain, last stripe's out-proj.
                # nb12's first 3 chunks (heads whose chains are already
                # normalized) overlap the chain-15 latency; its h=3 chunk
                # waits on chain_mul.  The aux ring (bufs=2) only allows one
                # partial to be in flight alongside the chain tile. ----
                while pvq:
                    pvq.popleft()()
                pr12 = emit_ph4_start(12, nchunks=KC - 1)
                chain_reduce(pending)
                chain_recip(pending)
                chain_bcast(pending)
                chain_mul(pending)
                emit_ph4_finish(12, pr12, kc0=KC - 1)
                for nb in range(13, 4 * NS):
                    emit_ph4_nb(nb)

    nc.compile()
    return nc


_NC = None


def _get_nc():
    global _NC
    if _NC is None:
        _NC = build_nc()
    return _NC


def kernel(F1, F2, W_qkv, b_qkv, W_proj, b_proj, _trace=False):
    F1 = np.asarray(F1)
    F2 = np.asarray(F2)
    F1T = np.ascontiguousarray(
        F1.astype(np.float16).transpose(0, 2, 1)
    )  # [B, C, N]
    F2T = np.ascontiguousarray(F2.astype(np.float16).transpose(0, 2, 1))
    Wh = np.ascontiguousarray(np.asarray(W_qkv).astype(np.float16))
    Wph = np.ascontiguousarray(np.asarray(W_proj).astype(np.float16))
    bqc = np.ascontiguousarray(
        np.asarray(b_qkv, dtype=np.float32).reshape(KC, P).T
    )
    bph = np.ascontiguousarray(
        np.asarray(b_proj, dtype=np.float32).reshape(1, C)
    )

    nc = _get_nc()
    in_maps = [
        {"F1T": F1T[b], "F2T": F2T[b], "Wqkv": Wh, "bqc": bqc,
         "Wproj": Wph, "bproj": bph}
        for b in range(B)
    ]
    res = run_bass_kernel_spmd(
        nc, in_maps, core_ids=list(range(B)), trace=_trace
    )
    out = np.stack([res.results[b]["OUT"] for b in range(B)], axis=0)
    if _trace:
        return out, res
    return out
